# revision 7
# baseline (speedup 1.0000x reference)
"""Trainium2 Bass kernel for the E74 fixed-decay delta-rule cell.

Reference computation (per batch b):
    proj = x @ W_kvq^T -> k, v, q   (T steps, D=1024 -> 3n, n=64)
    for t in 1..T:
        kh = k_t / (||k_t|| + eps)
        delta_t = v_t - S k h_t
        S = alpha*S + outer(delta_t, kh_t)
        out_t = (S q_t) * silu(S q_t)

Chunked reformulation (chunk size C, per chunk, entering state S0):
    A[t,s]   = alpha^{t-1-s} (kh_t . kh_s)  for s<t   (strictly lower CxC)
    R        = [alpha^{t-1} kh | v]                      [C, 2n]
    X        = (I+A)^{-1} R  (truncated Neumann, exact to fp32 for m~12-16
               since ||A|| ~ 0.3)  ->  W = X[:, :n], U = X[:, n:]
    Delta    = U - W S0^T
    S_C^T    = alpha^C S0^T + (alpha^{C-t} kh)^T Delta
    Sq       = alpha^t (q S0^T) + (Bm o (q kh^T)) Delta,  Bm[t,s]=alpha^{t-s}, s<=t
    out      = Sq * silu(Sq)

All matmuls arranged transpose-free:
  - Gram matrices are symmetric -> A^T comes from a host mask on the same Gram.
  - W is produced directly in transposed layout via one extra matmul against
    the second-to-last Horner iterate.
  - The state is carried as S^T only.

Sharding: batch B=16 over 8 cores (2 per core). W replicated. Forward only.
"""

import os
import sys

sys.path.insert(0, "/opt/trn_rl_repo")

import numpy as np

T_FULL = 4096
B_FULL = 16
D = 1024
N = 64
NCORES = 8
BLOC = B_FULL // NCORES  # 2
C = 128  # chunk size
ALPHA = 0.9
EPS = 1e-6
M_HORNER = int(os.environ.get("E74_M_HORNER", "14"))
NKD = D // 128  # 8 contraction chunks


def host_consts():
    """Host-precomputed constant tensors shared by all cores."""
    f32 = np.float32
    tv = np.arange(1, C + 1, dtype=np.float64)  # t = 1..C
    tt = tv[:, None] - tv[None, :]  # t - s
    # A mask, transposed: lt[s,t] = alpha^{t-1-s} for s<t else 0
    L = np.where(tt > 0, ALPHA ** (tt - 1), 0.0)
    lt = np.ascontiguousarray(L.T).astype(f32)
    # B mask, transposed: bmt[s,t] = alpha^{t-s} for s<=t else 0
    Bm = np.where(tt >= 0, ALPHA**tt, 0.0)
    bmt = np.ascontiguousarray(Bm.T).astype(f32)
    dec1c = (ALPHA ** (tv - 1)).astype(f32).reshape(C, 1)  # alpha^{t-1}
    dec1r = np.broadcast_to((ALPHA ** (tv - 1)).astype(f32).reshape(1, C),
                            (N, C)).copy()
    dec3c = (ALPHA ** (C - tv)).astype(f32).reshape(C, 1)  # alpha^{C-t}
    dec2r = np.broadcast_to((ALPHA**tv).astype(f32).reshape(1, C),
                            (N, C)).copy()  # alpha^t
    ident = np.eye(128, dtype=f32)
    return dict(lt=lt, bmt=bmt, dec1c=dec1c, dec1r=dec1r, dec3c=dec3c,
                dec2r=dec2r, ident=ident)


def build_nc(t_total=T_FULL, m_horner=M_HORNER):
    import concourse.bacc as bacc
    import concourse.mybir as mybir
    import concourse.tile as tile

    f32 = mybir.dt.float32
    AF = mybir.ActivationFunctionType
    nch = t_total // C
    aC = float(ALPHA**C)

    nc = bacc.Bacc("TRN2", target_bir_lowering=False, debug=False,
                   num_devices=NCORES)

    x_d = nc.dram_tensor("x_loc", [t_total, BLOC, D], f32, kind="ExternalInput").ap()
    s0t_d = nc.dram_tensor("s0t", [BLOC, N, N], f32, kind="ExternalInput").ap()
    wt_d = nc.dram_tensor("wt", [D, 3 * N], f32, kind="ExternalInput").ap()
    lt_d = nc.dram_tensor("lt", [C, C], f32, kind="ExternalInput").ap()
    bmt_d = nc.dram_tensor("bmt", [C, C], f32, kind="ExternalInput").ap()
    dec1c_d = nc.dram_tensor("dec1c", [C, 1], f32, kind="ExternalInput").ap()
    dec1r_d = nc.dram_tensor("dec1r", [N, C], f32, kind="ExternalInput").ap()
    dec3c_d = nc.dram_tensor("dec3c", [C, 1], f32, kind="ExternalInput").ap()
    dec2r_d = nc.dram_tensor("dec2r", [N, C], f32, kind="ExternalInput").ap()
    ident_d = nc.dram_tensor("ident", [128, 128], f32, kind="ExternalInput").ap()

    y_d = nc.dram_tensor("y", [t_total, BLOC, N], f32, kind="ExternalOutput").ap()
    sft_d = nc.dram_tensor("sft", [BLOC, N, N], f32, kind="ExternalOutput").ap()

    add = mybir.AluOpType.add
    mult = mybir.AluOpType.mult

    with tile.TileContext(nc) as tc:
        with (
            tc.tile_pool(name="consts", bufs=1) as cpool,
            tc.tile_pool(name="xin", bufs=3) as xpool,
            tc.tile_pool(name="xt", bufs=2) as xtpool,
            tc.tile_pool(name="work", bufs=3) as wpool,
            tc.tile_pool(name="horner", bufs=3) as hpool,
            tc.tile_pool(name="persist", bufs=1) as ppool,
            tc.tile_pool(name="outp", bufs=4) as opool,
            tc.tile_pool(name="psA", bufs=2, space="PSUM") as psA,
            tc.tile_pool(name="psB", bufs=2, space="PSUM") as psB,
        ):
            # ---- constants ----
            lt_t = cpool.tile([C, C], f32, tag="lt")
            bmt_t = cpool.tile([C, C], f32, tag="bmt")
            ident_t = cpool.tile([128, 128], f32, tag="ident")
            dec1c_t = cpool.tile([C, 1], f32, tag="dec1c")
            dec1r_t = cpool.tile([N, C], f32, tag="dec1r")
            dec3c_t = cpool.tile([C, 1], f32, tag="dec3c")
            dec2r_t = cpool.tile([N, C], f32, tag="dec2r")
            wt_t = cpool.tile([128, NKD, 3 * N], f32, tag="wt")
            nc.sync.dma_start(lt_t[:], lt_d[:])
            nc.sync.dma_start(bmt_t[:], bmt_d[:])
            nc.sync.dma_start(ident_t[:], ident_d[:])
            nc.sync.dma_start(dec1c_t[:], dec1c_d[:])
            nc.sync.dma_start(dec1r_t[:], dec1r_d[:])
            nc.sync.dma_start(dec3c_t[:], dec3c_d[:])
            nc.sync.dma_start(dec2r_t[:], dec2r_d[:])
            nc.sync.dma_start(
                wt_t[:], wt_d.rearrange("(j p) f -> p j f", p=128)
            )

            # ---- per-batch S^T state chains (all chunk-entry states kept) ----
            NS = 4  # ring depth for chunk-entry states
            s0t_store = [
                ppool.tile([N, NS * N], f32, tag=f"s0t_b{b}", name=f"s0t_b{b}")
                for b in range(BLOC)
            ]
            for b in range(BLOC):
                nc.sync.dma_start(s0t_store[b][:, 0:N], s0t_d[b])

            # persistent per-(chunk, batch) tensors
            X_p = {}    # [C, 2N]: cols 0:N become Delta, N:2N hold U
            WT_p = {}   # [N, C]
            KA2_p = {}  # [C, N]
            BT_p = {}   # [C, C]
            QTA_p = {}  # [N, C]

            dec1r_b = dec1r_t[:]
            dec2r_b = dec2r_t[:]

            # =========== PHASE P: per-chunk parallel precompute ============
            for g in range(nch):
                for b in range(BLOC):
                    t0 = g * C
                    xs = xpool.tile([128, D], f32, tag="xs")
                    nc.sync.dma_start(xs[:], x_d[t0 : t0 + C, b, :])

                    # transpose x tile: xt[:, j, :] = xs[:, 128j:128j+128]^T
                    xt = xtpool.tile([128, NKD, 128], f32, tag="xt")
                    for j in range(NKD):
                        pt = psA.tile([128, 128], f32, tag="pt")
                        nc.tensor.transpose(
                            pt[:], xs[:, j * 128 : (j + 1) * 128], ident_t[:]
                        )
                        if j % 2 == 0:
                            nc.vector.tensor_copy(xt[:, j, :], pt[:])
                        else:
                            nc.scalar.copy(xt[:, j, :], pt[:])

                    # projection: proj[t, f] = sum_d x[t,d] wt[d,f]
                    pp = psA.tile([128, 3 * N], f32, tag="pp")
                    for j in range(NKD):
                        nc.tensor.matmul(
                            pp[:], xt[:, j, :], wt_t[:, j, :],
                            start=(j == 0), stop=(j == NKD - 1),
                        )

                    # k normalization (rows of k: [C, N])
                    ksq = wpool.tile([C, N], f32, tag="ksq")
                    s2 = wpool.tile([C, 1], f32, tag="s2")
                    nc.scalar.activation(ksq[:], pp[:, 0:N], AF.Square,
                                         accum_out=s2[:])
                    nrm = wpool.tile([C, 1], f32, tag="nrm")
                    nc.scalar.sqrt(nrm[:], s2[:])
                    nc.vector.tensor_scalar_add(nrm[:], nrm[:], EPS)
                    rinv = wpool.tile([C, 1], f32, tag="rinv")
                    nc.vector.reciprocal(rinv[:], nrm[:])
                    kh = wpool.tile([C, N], f32, tag="kh")
                    nc.vector.tensor_scalar_mul(kh[:], pp[:, 0:N], rinv[:])
                    qs = wpool.tile([C, N], f32, tag="qs")
                    nc.scalar.copy(qs[:], pp[:, 2 * N : 3 * N])

                    # transposes of kh and q  ->  [N, C]
                    ptk = psA.tile([N, C], f32, tag="pt", name="ptk")
                    nc.tensor.transpose(ptk[:], kh[:], ident_t[:])
                    khT = wpool.tile([N, C], f32, tag="khT")
                    nc.vector.tensor_copy(khT[:], ptk[:])
                    ptq = psA.tile([N, C], f32, tag="pt", name="ptq")
                    nc.tensor.transpose(ptq[:], qs[:], ident_t[:])
                    qTa = ppool.tile([N, C], f32, tag=f"qta_{g}_{b}")
                    nc.vector.tensor_tensor(qTa[:], ptq[:], dec2r_b, op=mult)
                    qTs = wpool.tile([N, C], f32, tag="qTs")
                    nc.scalar.copy(qTs[:], ptq[:])

                    # Gram matrices
                    pg = psA.tile([C, C], f32, tag="pt", name="pg")
                    nc.tensor.matmul(pg[:], khT[:], khT[:])
                    at = wpool.tile([C, C], f32, tag="at")
                    nc.vector.tensor_tensor(at[:], pg[:], lt_t[:], op=mult)
                    pg2 = psA.tile([C, C], f32, tag="pt", name="pg2")
                    nc.tensor.matmul(pg2[:], khT[:], qTs[:])
                    bt = ppool.tile([C, C], f32, tag=f"bt_{g}_{b}")
                    nc.vector.tensor_tensor(bt[:], pg2[:], bmt_t[:], op=mult)

                    # Horner solve X = (I+A)^{-1} [dec1*kh | v]
                    r_t = hpool.tile([C, 2 * N], f32, tag="hr")
                    nc.vector.tensor_scalar_mul(r_t[:, 0:N], kh[:], dec1c_t[:])
                    nc.scalar.copy(r_t[:, N : 2 * N], pp[:, N : 2 * N])

                    xlast = ppool.tile([C, 2 * N], f32, tag=f"x_{g}_{b}")
                    xprev = r_t
                    for it in range(1, m_horner):
                        dst = (
                            xlast
                            if it == m_horner - 1
                            else hpool.tile([C, 2 * N], f32,
                                            tag=f"hx{it % 2}", name="hx")
                        )
                        ph = psB.tile([C, 2 * N], f32, tag="ph")
                        nc.tensor.matmul(ph[:], at[:], xprev[:])
                        if it == m_horner - 1:
                            # transposed W output from the same iterate
                            pw = psB.tile([N, C], f32, tag="ph", name="pw")
                            nc.tensor.matmul(pw[:], xprev[:, 0:N], at[:])
                            rwt = wpool.tile([N, C], f32, tag="rwt")
                            nc.vector.tensor_tensor(
                                rwt[:], khT[:], dec1r_b, op=mult
                            )
                            wt_p = ppool.tile([N, C], f32, tag=f"wt_{g}_{b}")
                            nc.vector.tensor_sub(wt_p[:], rwt[:], pw[:])
                            WT_p[(g, b)] = wt_p
                        nc.vector.tensor_sub(dst[:], r_t[:], ph[:])
                        xprev = dst
                    X_p[(g, b)] = xlast

                    ka2 = ppool.tile([C, N], f32, tag=f"ka2_{g}_{b}")
                    nc.vector.tensor_scalar_mul(ka2[:], kh[:], dec3c_t[:])
                    KA2_p[(g, b)] = ka2
                    BT_p[(g, b)] = bt
                    QTA_p[(g, b)] = qTa

            # ======== PHASE S+O: sequential state pass + outputs ==========
            for g in range(nch):
                for b in range(BLOC):
                    t0 = g * C
                    s0t_g = s0t_store[b][:, (g % NS) * N : (g % NS + 1) * N]
                    s0t_n = s0t_store[b][
                        :, ((g + 1) % NS) * N : ((g + 1) % NS + 1) * N
                    ]
                    xl = X_p[(g, b)]

                    # Delta = U - W S0^T   (into xl[:, 0:N])
                    p1 = psB.tile([C, N], f32, tag="po", name="p1")
                    nc.tensor.matmul(p1[:], WT_p[(g, b)][:], s0t_g)
                    nc.vector.tensor_sub(xl[:, 0:N], xl[:, N : 2 * N], p1[:])

                    # S_{g+1}^T = aC * S_g^T + Ka2^T Delta
                    ps = psB.tile([N, N], f32, tag="po", name="ps")
                    nc.tensor.matmul(ps[:], KA2_p[(g, b)][:], xl[:, 0:N])
                    nc.vector.scalar_tensor_tensor(
                        s0t_n, s0t_g, aC, ps[:], op0=mult, op1=add
                    )

                    # Sq = qTa^T S_g^T + BT^T Delta ; out = Sq * silu(Sq)
                    pq = psB.tile([C, N], f32, tag="po", name="pq")
                    nc.tensor.matmul(pq[:], QTA_p[(g, b)][:], s0t_g, start=True,
                                     stop=False)
                    nc.tensor.matmul(pq[:], BT_p[(g, b)][:], xl[:, 0:N],
                                     start=False, stop=True)
                    sig = opool.tile([C, N], f32, tag="sig")
                    nc.scalar.activation(sig[:], pq[:], AF.Sigmoid)
                    sq2 = opool.tile([C, N], f32, tag="sq2")
                    nc.scalar.square(sq2[:], pq[:])
                    outv = opool.tile([C, N], f32, tag="outv")
                    nc.vector.tensor_mul(outv[:], sq2[:], sig[:])
                    nc.sync.dma_start(y_d[t0 : t0 + C, b, :], outv[:])

            # final states
            for b in range(BLOC):
                nc.sync.dma_start(
                    sft_d[b],
                    s0t_store[b][:, (nch % NS) * N : (nch % NS + 1) * N],
                )

    nc.compile()
    return nc


_NC_CACHE = {}
TRACE = False
LAST_RESULT = None


def _get_nc():
    key = (T_FULL, M_HORNER)
    if key not in _NC_CACHE:
        _NC_CACHE[key] = build_nc()
    return _NC_CACHE[key]


def kernel(x, S, W_kvq):
    """Full-input entry point: shards over batch across 8 cores."""
    from concourse.bass_utils import run_bass_kernel_spmd

    x = np.asarray(x)
    S = np.asarray(S)
    W_kvq = np.asarray(W_kvq)
    nc = _get_nc()
    consts = host_consts()
    wt = np.ascontiguousarray(W_kvq.T.astype(np.float32))  # [D, 3n]

    in_maps = []
    for c in range(NCORES):
        bsl = slice(c * BLOC, (c + 1) * BLOC)
        m = {
            "x_loc": np.ascontiguousarray(x[:, bsl, :]),
            "s0t": np.ascontiguousarray(
                np.transpose(S[bsl], (0, 2, 1))
            ),
            "wt": wt,
        }
        m.update(consts)
        in_maps.append(m)

    global LAST_RESULT
    res = run_bass_kernel_spmd(nc, in_maps, list(range(NCORES)), trace=TRACE)
    LAST_RESULT = res
    outs = np.concatenate(
        [res.results[c]["y"] for c in range(NCORES)], axis=1
    )
    s_final = np.concatenate(
        [np.transpose(res.results[c]["sft"], (0, 2, 1)) for c in range(NCORES)],
        axis=0,
    )
    return outs, s_final
